# revision 15
# baseline (speedup 1.0000x reference)
"""PixelContrastMorphologyLoss on 8 Trainium2 NeuronCores.

Data-parallel over batch: each core handles 2 of 16 images. Per image, on
device: per-class top-100 selection via kth_largest threshold; selected-pixel
ranks via prefix-sum matmuls (strict-lower-triangular ones); a one-hot
selection matrix S built per 128-pixel chunk on DVE; feature "gather" as
A = sum_k F_k^T S_k on the PE (exact: products are f*1 or f*0); then the
224x224 pixel Gram matrix. Host feeds features pre-transposed to
[pixel, channel] so chunk pixels land on SBUF partitions. The tiny
contrastive epilogue (~0.5 MFLOP on 200x200 blocks) runs on host in f32
with exact IEEE semantics, mirroring the reference op-for-op. Loss is
invariant to within-class pixel ordering, so rank order (column-major) need
not match the reference's row-major order.
"""

import numpy as np

import concourse.bacc as bacc
import concourse.bass as bass
import concourse.tile as tile
from concourse import mybir
from concourse.bass_utils import run_bass_kernel_spmd

B, C, H, W = 16, 256, 128, 128
HW = H * W  # 16384
N_CORES = 8
IMGS_PER_CORE = B // N_CORES  # 2
CLASSES = (1, 2)
N_VIEW = 100
TEMP = 0.1
BASE_TEMP = 0.07
SLOTS = 112  # class-2 column offset in the 224-wide one-hot
NSL = 2 * SLOTS  # 224
# kth_largest: all 16384 lanes valid (masked scores = 0.0), so
# k_adj = floor((1-q)*16383) = 99 -> out[0,1] = desc[100], the 101st-largest
# score = exact top-100 cut (score values are distinct).
QUANTILE = 1.0 - 0.00607
KTH_K = 102
CHUNKS = 128  # 16384 / 128
CPL = 4  # chunks per feats DMA load

_CACHE = {}


def _build_kernel():
    if "nc" in _CACHE:
        return _CACHE["nc"]
    fp32 = mybir.dt.float32
    nc = bacc.Bacc(None, target_bir_lowering=False, debug=True)
    featsT_d = nc.dram_tensor("featsT", [IMGS_PER_CORE, HW, C], fp32,
                              kind="ExternalInput")
    mask_d = nc.dram_tensor("mask", [IMGS_PER_CORE, H, W], mybir.dt.int32,
                            kind="ExternalInput")
    noise_d = nc.dram_tensor("noise", [IMGS_PER_CORE, H, W], fp32,
                             kind="ExternalInput")
    lt_d = nc.dram_tensor("lt", [128, 128], fp32, kind="ExternalInput")
    ones128_d = nc.dram_tensor("ones128", [128, 1], fp32, kind="ExternalInput")
    ones1_d = nc.dram_tensor("ones1", [1, 1], fp32, kind="ExternalInput")
    onesrow_d = nc.dram_tensor("onesrow", [1, 128], fp32, kind="ExternalInput")
    iota224_d = nc.dram_tensor("iota224", [128, NSL], fp32,
                               kind="ExternalInput")
    ones224_d = nc.dram_tensor("ones224", [128, NSL], fp32,
                               kind="ExternalInput")
    gram_d = nc.dram_tensor("gram", [IMGS_PER_CORE, NSL, NSL], fp32,
                            kind="ExternalOutput")

    with tile.TileContext(nc) as tc:
        with (
            tc.tile_pool(name="const", bufs=1) as const_pool,
            tc.tile_pool(name="mn", bufs=2) as mn_pool,
            tc.tile_pool(name="score", bufs=2) as score_pool,
            tc.tile_pool(name="small", bufs=3) as small_pool,
            tc.tile_pool(name="rm", bufs=2) as rm_pool,
            tc.tile_pool(name="feat", bufs=3) as feat_pool,
            tc.tile_pool(name="sel1h", bufs=4) as s_pool,
            tc.tile_pool(name="av", bufs=2) as a_pool,
            tc.tile_pool(name="gout", bufs=2) as g_pool,
            tc.tile_pool(name="psA", bufs=1, space=bass.MemorySpace.PSUM) as psA,
            tc.tile_pool(name="psS", bufs=1, space=bass.MemorySpace.PSUM) as psS,
            tc.tile_pool(name="psG", bufs=2, space=bass.MemorySpace.PSUM) as psG,
        ):
            lt = const_pool.tile([128, 128], fp32)
            ones128 = const_pool.tile([128, 1], fp32)
            ones1 = const_pool.tile([1, 1], fp32)
            onesrow = const_pool.tile([1, 128], fp32)
            iota224 = const_pool.tile([128, NSL], fp32)
            ones224 = const_pool.tile([128, NSL], fp32)
            nc.scalar.dma_start(lt[:], lt_d[:])
            nc.scalar.dma_start(ones128[:], ones128_d[:])
            nc.scalar.dma_start(ones1[:], ones1_d[:])
            nc.scalar.dma_start(onesrow[:], onesrow_d[:])
            nc.scalar.dma_start(iota224[:], iota224_d[:])
            nc.scalar.dma_start(ones224[:], ones224_d[:])

            for b in range(IMGS_PER_CORE):
                m_sb = mn_pool.tile([H, W], mybir.dt.int32)
                n_sb = mn_pool.tile([H, W], fp32)
                nc.scalar.dma_start(m_sb[:], mask_d[b])
                nc.scalar.dma_start(n_sb[:], noise_d[b])

                rms = []
                for ci, cls in enumerate(CLASSES):
                    # score = (mask == cls) ? noise : 0
                    score = score_pool.tile([H, W], fp32)
                    nc.vector.scalar_tensor_tensor(
                        score[:], m_sb[:], float(cls), n_sb[:],
                        op0=mybir.AluOpType.is_equal,
                        op1=mybir.AluOpType.mult)
                    kth = small_pool.tile([1, 2], fp32)
                    nc.gpsimd.kth_largest(kth[:], score[:], n_per_lane=W,
                                          k=KTH_K, quantile=QUANTILE)
                    thr = small_pool.tile([128, 2], fp32)
                    nc.gpsimd.partition_broadcast(thr[:], kth[:])
                    # sel = score > thr : exactly 100 ones
                    sel = score_pool.tile([H, W], fp32)
                    nc.vector.tensor_scalar(
                        sel[:], score[:], thr[:, 1:2], None,
                        op0=mybir.AluOpType.is_gt)
                    # rank[r,c] = #sel with c'<c or (c'==c and r'<r)
                    rank_ps = psS.tile([128, 128], fp32)
                    nc.tensor.matmul(rank_ps[:], lt[:], sel[:],
                                     start=True, stop=False)
                    colt_ps = psS.tile([1, 128], fp32)
                    nc.tensor.matmul(colt_ps[:], ones128[:], sel[:],
                                     start=True, stop=True)
                    colt = small_pool.tile([1, 128], fp32)
                    nc.vector.tensor_copy(colt[:], colt_ps[:])
                    coltt_ps = psS.tile([128, 1], fp32)
                    nc.tensor.matmul(coltt_ps[:], colt[:], ones1[:],
                                     start=True, stop=True)
                    coltt = small_pool.tile([128, 1], fp32)
                    nc.vector.tensor_copy(coltt[:], coltt_ps[:])
                    off_ps = psS.tile([1, 128], fp32)
                    nc.tensor.matmul(off_ps[:], coltt[:], lt[:],
                                     start=True, stop=True)
                    off = small_pool.tile([1, 128], fp32)
                    nc.vector.tensor_copy(off[:], off_ps[:])
                    # rank += off (broadcast along partitions via outer prod)
                    nc.tensor.matmul(rank_ps[:], onesrow[:], off[:],
                                     start=False, stop=True)
                    # rm = sel ? rank + class_off : rank + class_off + 1000
                    rank = rm_pool.tile([128, 128], fp32)
                    nc.vector.tensor_scalar_add(rank[:], rank_ps[:],
                                                1000.0 + ci * SLOTS)
                    rm_c = rm_pool.tile([128, 128], fp32)
                    nc.vector.scalar_tensor_tensor(
                        rm_c[:], sel[:], -1000.0, rank[:],
                        op0=mybir.AluOpType.mult, op1=mybir.AluOpType.add)
                    rms.append(rm_c)
                # classes are disjoint: min merges both one-hot targets
                rm = rm_pool.tile([128, 128], fp32)
                nc.vector.tensor_tensor(rm[:], rms[0][:], rms[1][:],
                                        op=mybir.AluOpType.min)

                a_lo_ps = psA.tile([128, NSL], fp32)
                a_hi_ps = psA.tile([128, NSL], fp32)
                for kb in range(CHUNKS // CPL):
                    f_t = feat_pool.tile([128, CPL * C], fp32)
                    eng = nc.sync if kb % 2 == 0 else nc.scalar
                    src = featsT_d[b, kb * CPL * 128:(kb + 1) * CPL * 128, :]
                    eng.dma_start(
                        f_t[:].rearrange("p (j ch) -> p j ch", j=CPL),
                        src.rearrange("(j p) ch -> p j ch", j=CPL))
                    for j in range(CPL):
                        k = kb * CPL + j
                        s_t = s_pool.tile([128, NSL], fp32)
                        nc.vector.tensor_scalar(
                            s_t[:], iota224[:], rm[:, k:k + 1], None,
                            op0=mybir.AluOpType.is_equal)
                        nc.tensor.matmul(
                            a_lo_ps[:], f_t[:, j * C:j * C + 128], s_t[:],
                            start=(k == 0), stop=(k == CHUNKS - 1))
                        nc.tensor.matmul(
                            a_hi_ps[:], f_t[:, j * C + 128:(j + 1) * C],
                            s_t[:], start=(k == 0), stop=(k == CHUNKS - 1))
                a_lo = a_pool.tile([128, NSL], fp32)
                a_hi = a_pool.tile([128, NSL], fp32)
                nc.scalar.copy(a_lo[:], a_lo_ps[:])
                nc.scalar.copy(a_hi[:], a_hi_ps[:])
                for m0, msz in ((0, 128), (128, 96)):
                    acc = psG.tile([msz, NSL], fp32)
                    nc.tensor.matmul(acc[:], a_lo[:, m0:m0 + msz], a_lo[:],
                                     start=True, stop=False)
                    nc.tensor.matmul(acc[:], a_hi[:, m0:m0 + msz], a_hi[:],
                                     start=False, stop=True)
                    g_sb = g_pool.tile([msz, NSL], fp32)
                    nc.vector.tensor_copy(g_sb[:], acc[:])
                    nc.sync.dma_start(gram_d[b, m0:m0 + msz, :], g_sb[:])

    nc.finalize()
    _CACHE["nc"] = nc
    return nc


def _host_inputs():
    if "host" in _CACHE:
        return _CACHE["host"]
    lt = np.triu(np.ones((128, 128), np.float32), 1)
    iota = np.broadcast_to(np.arange(NSL, dtype=np.float32),
                           (128, NSL)).copy()
    _CACHE["host"] = {
        "lt": lt,
        "ones128": np.ones((128, 1), np.float32),
        "ones1": np.ones((1, 1), np.float32),
        "onesrow": np.ones((1, 128), np.float32),
        "iota224": iota,
        "ones224": np.ones((128, NSL), np.float32),
    }
    return _CACHE["host"]


def _epilogue(gram: np.ndarray) -> np.ndarray:
    """Reference contrastive loss from per-image pixel Gram matrices.

    gram: [B, 224, 224] f32; valid anchors at [0:100] (class 1) and
    [112:212] (class 2). Mirrors reference ops in f32 (incl. 0*inf=nan)."""
    keep = np.r_[0:N_VIEW, SLOTS:SLOTS + N_VIEW]
    g = gram[:, keep][:, :, keep].astype(np.float32)  # [B, 200, 200]
    lab = np.repeat(np.asarray(CLASSES, np.float32), N_VIEW)
    pos = (lab[:, None] == lab[None, :]).astype(np.float32)
    n = 2 * N_VIEW
    with np.errstate(all="ignore"):
        logits = (g / np.float32(TEMP)).astype(np.float32)
        logits = logits - logits.max(axis=2, keepdims=True)
        mask = pos * (1.0 - np.eye(n, dtype=np.float32))
        exp_logits = np.exp(logits)
        neg_sum = (exp_logits * (1.0 - pos)).sum(axis=2, keepdims=True,
                                                 dtype=np.float32)
        log_prob = logits - np.log(exp_logits + neg_sum)
        mlpp = (mask * log_prob).sum(axis=2) / mask.sum(axis=1)
        losses = -(TEMP / BASE_TEMP) * mlpp.mean(axis=1)
        out = losses.mean()
    return np.float32(out)


def kernel(feats, mask_within, mask_boundary, predict, sel_noise):
    nc = _build_kernel()
    consts = _host_inputs()
    feats = np.asarray(feats, dtype=np.float32)
    in_maps = []
    for core in range(N_CORES):
        lo = core * IMGS_PER_CORE
        hi = lo + IMGS_PER_CORE
        ft = np.ascontiguousarray(
            feats[lo:hi].reshape(IMGS_PER_CORE, C, H, W)
            .transpose(0, 3, 2, 1)).reshape(IMGS_PER_CORE, HW, C)
        in_maps.append({
            "featsT": ft,
            "mask": np.ascontiguousarray(mask_within[lo:hi], dtype=np.int32),
            "noise": np.ascontiguousarray(
                sel_noise[lo:hi], dtype=np.float32).reshape(
                    IMGS_PER_CORE, H, W),
            **consts,
        })
    res = run_bass_kernel_spmd(nc, in_maps, list(range(N_CORES)),
                               **_CACHE.get("run_kwargs", {}))
    _CACHE["last_res"] = res
    grams = np.concatenate([r["gram"] for r in res.results], axis=0)
    return np.asarray(_epilogue(grams))


# revision 17
# speedup vs baseline: 2.3525x; 2.3525x over previous
"""PixelContrastMorphologyLoss on 8 Trainium2 NeuronCores.

Data-parallel over batch: each core handles 2 of 16 images. Per image, on
device: per-class top-100 selection thresholds found by a vectorized
bisection (30 fixed iterations over all 4 image/class pairs at once, DVE +
PE only — no GPSIMD ucode); selected-pixel ranks via prefix-sum matmuls
(strict-upper-triangular ones); a one-hot selection matrix S built per
128-pixel chunk on DVE; feature "gather" as A = sum_k F_k^T S_k on the PE
in bf16 (products are f*1 or f*0, so A == bf16(feats) exactly); then the
224x224 pixel Gram matrix (bf16 inputs, fp32 PSUM accumulate). Host feeds
features pre-transposed to [pixel, channel] so chunk pixels land on SBUF
partitions. The tiny contrastive epilogue (~0.5 MFLOP on 200x200 blocks)
runs on host in f32 with exact IEEE semantics, mirroring the reference
op-for-op (its result is nan by construction: diagonal dominance drives
off-diagonal exp to 0 -> log(0) -> 0*inf). Loss is invariant to
within-class pixel ordering, so rank order (column-major) need not match
the reference's row-major order.
"""

import numpy as np

import concourse.bacc as bacc
import concourse.bass as bass
import concourse.tile as tile
from concourse import mybir
from concourse.bass_utils import run_bass_kernel_spmd

B, C, H, W = 16, 256, 128, 128
HW = H * W  # 16384
N_CORES = 8
IMGS_PER_CORE = B // N_CORES  # 2
CLASSES = (1, 2)
N_VIEW = 100
TEMP = 0.1
BASE_TEMP = 0.07
SLOTS = 112  # class-2 column offset in the 224-wide one-hot
NSL = 2 * SLOTS  # 224
CHUNKS = 128  # 16384 / 128
CPL = 4  # chunks per feats DMA load
# bisection for the 101st-largest score: noise is f32 in [0,1), distinct
# adjacent order stats differ by >= 2^-25, and the interval halves each
# step (stalling harmlessly at 1-ulp width), so 30 iterations always land
# the threshold strictly between the 100th and 101st values.
ITERS = 30

_CACHE = {}


def _build_kernel():
    if "nc" in _CACHE:
        return _CACHE["nc"]
    fp32 = mybir.dt.float32
    bf16 = mybir.dt.bfloat16
    alu = mybir.AluOpType
    nc = bacc.Bacc(None, target_bir_lowering=False, debug=True)
    featsT_d = nc.dram_tensor("featsT", [IMGS_PER_CORE, HW, C], fp32,
                              kind="ExternalInput")
    mask_d = nc.dram_tensor("mask", [IMGS_PER_CORE, H, W], mybir.dt.int32,
                            kind="ExternalInput")
    noise_d = nc.dram_tensor("noise", [IMGS_PER_CORE, H, W], fp32,
                             kind="ExternalInput")
    lt_d = nc.dram_tensor("lt", [128, 128], fp32, kind="ExternalInput")
    ones128_d = nc.dram_tensor("ones128", [128, 1], fp32, kind="ExternalInput")
    ones1_d = nc.dram_tensor("ones1", [1, 1], fp32, kind="ExternalInput")
    onesrow_d = nc.dram_tensor("onesrow", [1, 128], fp32, kind="ExternalInput")
    iota224_d = nc.dram_tensor("iota224", [128, NSL], fp32,
                               kind="ExternalInput")
    gram_d = nc.dram_tensor("gram", [IMGS_PER_CORE, NSL, NSL], fp32,
                            kind="ExternalOutput")

    with tile.TileContext(nc) as tc:
        with (
            tc.tile_pool(name="const", bufs=1) as const_pool,
            tc.tile_pool(name="mn", bufs=2) as mn_pool,
            tc.tile_pool(name="sc", bufs=1) as sc_pool,
            tc.tile_pool(name="small", bufs=2) as small_pool,
            tc.tile_pool(name="rm", bufs=2) as rm_pool,
            tc.tile_pool(name="feat", bufs=20) as feat_pool,
            tc.tile_pool(name="fbf", bufs=8) as fbf_pool,
            tc.tile_pool(name="sel1h", bufs=4) as s_pool,
            tc.tile_pool(name="av", bufs=2) as a_pool,
            tc.tile_pool(name="gout", bufs=2) as g_pool,
            tc.tile_pool(name="psA", bufs=1, space=bass.MemorySpace.PSUM) as psA,
            tc.tile_pool(name="psS", bufs=1, space=bass.MemorySpace.PSUM) as psS,
            tc.tile_pool(name="psG", bufs=2, space=bass.MemorySpace.PSUM) as psG,
        ):
            lt = const_pool.tile([128, 128], fp32)
            ones128 = const_pool.tile([128, 1], fp32)
            ones1 = const_pool.tile([1, 1], fp32)
            onesrow = const_pool.tile([1, 128], fp32)
            iota224 = const_pool.tile([128, NSL], fp32)
            nc.scalar.dma_start(lt[:], lt_d[:])
            nc.scalar.dma_start(ones128[:], ones128_d[:])
            nc.scalar.dma_start(ones1[:], ones1_d[:])
            nc.scalar.dma_start(onesrow[:], onesrow_d[:])
            nc.scalar.dma_start(iota224[:], iota224_d[:])

            # scores for all 4 (image, class) pairs side by side: block p4
            scores4 = sc_pool.tile([128, 4 * W], fp32)
            for b in range(IMGS_PER_CORE):
                m_sb = mn_pool.tile([H, W], mybir.dt.int32)
                n_sb = mn_pool.tile([H, W], fp32)
                nc.scalar.dma_start(m_sb[:], mask_d[b])
                nc.scalar.dma_start(n_sb[:], noise_d[b])
                for ci, cls in enumerate(CLASSES):
                    p4 = b * 2 + ci
                    nc.vector.scalar_tensor_tensor(
                        scores4[:, p4 * W:(p4 + 1) * W], m_sb[:], float(cls),
                        n_sb[:], op0=alu.is_equal, op1=alu.mult)

            # vectorized bisection: hi converges onto a threshold with
            # exactly 100 scores above it, per pair
            lo = small_pool.tile([1, 4], fp32, tag="lo", bufs=2)
            hi = small_pool.tile([1, 4], fp32, tag="hi", bufs=2)
            nc.vector.tensor_scalar(lo[:], onesrow[:, 0:4], 0.0, None,
                                    op0=alu.mult)
            nc.vector.tensor_copy(hi[:], onesrow[:, 0:4])
            for _ in range(ITERS):
                mid = small_pool.tile([1, 4], fp32, tag="mid", bufs=2)
                nc.vector.tensor_tensor(mid[:], lo[:], hi[:], op=alu.add)
                nc.vector.tensor_scalar(mid[:], mid[:], 0.5, None,
                                        op0=alu.mult)
                mid_ps = psS.tile([128, 4], fp32, tag="tmp_ps", bufs=2)
                nc.tensor.matmul(mid_ps[:], onesrow[:], mid[:],
                                 start=True, stop=True)
                mid128 = small_pool.tile([128, 4], fp32, tag="mid128", bufs=2)
                nc.vector.tensor_copy(mid128[:], mid_ps[:])
                gt = small_pool.tile([128, 4 * W], fp32, tag="gt", bufs=2)
                rowsum = small_pool.tile([128, 4], fp32, tag="rowsum", bufs=2)
                for p4 in range(4):
                    # reduce form of TensorScalarPtr requires both ALU ops
                    nc.vector.tensor_scalar(
                        gt[:, p4 * W:(p4 + 1) * W],
                        scores4[:, p4 * W:(p4 + 1) * W],
                        mid128[:, p4:p4 + 1], 1.0, op0=alu.is_gt,
                        op1=alu.mult,
                        accum_out=rowsum[:, p4:p4 + 1])
                cnt_ps = psS.tile([1, 4], fp32, tag="tmp_ps", bufs=2)
                nc.tensor.matmul(cnt_ps[:], ones128[:], rowsum[:],
                                 start=True, stop=True)
                cnt = small_pool.tile([1, 4], fp32, tag="cnt", bufs=2)
                nc.vector.tensor_copy(cnt[:], cnt_ps[:])
                cond = small_pool.tile([1, 4], fp32, tag="cond", bufs=2)
                nc.vector.tensor_scalar(cond[:], cnt[:], 100.5, None,
                                        op0=alu.is_gt)
                # lo' = lo + cond*(mid-lo);  hi' = hi - (1-cond)*(hi-mid)
                d = small_pool.tile([1, 4], fp32, tag="d", bufs=2)
                nc.vector.tensor_tensor(d[:], mid[:], lo[:], op=alu.subtract)
                nc.vector.tensor_tensor(d[:], cond[:], d[:], op=alu.mult)
                lo2 = small_pool.tile([1, 4], fp32, tag="lo", bufs=2)
                nc.vector.tensor_tensor(lo2[:], lo[:], d[:], op=alu.add)
                e = small_pool.tile([1, 4], fp32, tag="e", bufs=2)
                nc.vector.tensor_tensor(e[:], hi[:], mid[:], op=alu.subtract)
                notc = small_pool.tile([1, 4], fp32, tag="notc", bufs=2)
                nc.vector.tensor_scalar(notc[:], cond[:], -1.0, 1.0,
                                        op0=alu.mult, op1=alu.add)
                nc.vector.tensor_tensor(e[:], notc[:], e[:], op=alu.mult)
                hi2 = small_pool.tile([1, 4], fp32, tag="hi", bufs=2)
                nc.vector.tensor_tensor(hi2[:], hi[:], e[:], op=alu.subtract)
                lo, hi = lo2, hi2

            for b in range(IMGS_PER_CORE):
                rms = []
                for ci, cls in enumerate(CLASSES):
                    p4 = b * 2 + ci
                    thr_ps = psS.tile([128, 1], fp32, tag="tmp_ps", bufs=2)
                    nc.tensor.matmul(thr_ps[:], onesrow[:],
                                     hi[0:1, p4:p4 + 1], start=True, stop=True)
                    thr128 = small_pool.tile([128, 1], fp32, tag="thr128",
                                             bufs=2)
                    nc.vector.tensor_copy(thr128[:], thr_ps[:])
                    # sel = score > thr : exactly 100 ones
                    sel = small_pool.tile([H, W], fp32, tag="sel", bufs=2)
                    nc.vector.tensor_scalar(
                        sel[:], scores4[:, p4 * W:(p4 + 1) * W],
                        thr128[:, 0:1], None, op0=alu.is_gt)
                    # rank[r,c] = #sel with c'<c or (c'==c and r'<r)
                    rank_ps = psS.tile([128, 128], fp32, tag="rank_ps",
                                       bufs=1)
                    nc.tensor.matmul(rank_ps[:], lt[:], sel[:],
                                     start=True, stop=False)
                    colt_ps = psS.tile([1, 128], fp32, tag="tmp_ps", bufs=2)
                    nc.tensor.matmul(colt_ps[:], ones128[:], sel[:],
                                     start=True, stop=True)
                    colt = small_pool.tile([1, 128], fp32, tag="colt", bufs=2)
                    nc.vector.tensor_copy(colt[:], colt_ps[:])
                    coltt_ps = psS.tile([128, 1], fp32, tag="tmp_ps", bufs=2)
                    nc.tensor.matmul(coltt_ps[:], colt[:], ones1[:],
                                     start=True, stop=True)
                    coltt = small_pool.tile([128, 1], fp32, tag="coltt",
                                            bufs=2)
                    nc.vector.tensor_copy(coltt[:], coltt_ps[:])
                    off_ps = psS.tile([1, 128], fp32, tag="tmp_ps", bufs=2)
                    nc.tensor.matmul(off_ps[:], coltt[:], lt[:],
                                     start=True, stop=True)
                    off = small_pool.tile([1, 128], fp32, tag="off", bufs=2)
                    nc.vector.tensor_copy(off[:], off_ps[:])
                    # rank += off (broadcast along partitions via outer prod)
                    nc.tensor.matmul(rank_ps[:], onesrow[:], off[:],
                                     start=False, stop=True)
                    # rm = sel ? rank + class_off : rank + class_off + 1000
                    rank = rm_pool.tile([128, 128], fp32)
                    nc.vector.tensor_scalar_add(rank[:], rank_ps[:],
                                                1000.0 + ci * SLOTS)
                    rm_c = rm_pool.tile([128, 128], fp32)
                    nc.vector.scalar_tensor_tensor(
                        rm_c[:], sel[:], -1000.0, rank[:],
                        op0=alu.mult, op1=alu.add)
                    rms.append(rm_c)
                # classes are disjoint: min merges both one-hot targets
                rm = rm_pool.tile([128, 128], fp32)
                nc.vector.tensor_tensor(rm[:], rms[0][:], rms[1][:],
                                        op=alu.min)

                a_lo_ps = psA.tile([128, NSL], fp32)
                a_hi_ps = psA.tile([128, NSL], fp32)
                for kb in range(CHUNKS // CPL):
                    f_t = feat_pool.tile([128, CPL * C], fp32)
                    eng = (nc.sync, nc.scalar, nc.gpsimd)[kb % 3]
                    src = featsT_d[b, kb * CPL * 128:(kb + 1) * CPL * 128, :]
                    eng.dma_start(
                        f_t[:].rearrange("p (j ch) -> p j ch", j=CPL),
                        src.rearrange("(j p) ch -> p j ch", j=CPL))
                    f_bf = fbf_pool.tile([128, CPL * C], bf16)
                    nc.scalar.copy(f_bf[:], f_t[:])
                    for j in range(CPL):
                        k = kb * CPL + j
                        s_t = s_pool.tile([128, NSL], bf16)
                        nc.vector.tensor_scalar(
                            s_t[:], iota224[:], rm[:, k:k + 1], None,
                            op0=alu.is_equal)
                        nc.tensor.matmul(
                            a_lo_ps[:], f_bf[:, j * C:j * C + 128], s_t[:],
                            start=(k == 0), stop=(k == CHUNKS - 1))
                        nc.tensor.matmul(
                            a_hi_ps[:], f_bf[:, j * C + 128:(j + 1) * C],
                            s_t[:], start=(k == 0), stop=(k == CHUNKS - 1))
                # A holds bf16(feats) values exactly; cast back for the Gram
                a_lo = a_pool.tile([128, NSL], bf16)
                a_hi = a_pool.tile([128, NSL], bf16)
                nc.scalar.copy(a_lo[:], a_lo_ps[:])
                nc.scalar.copy(a_hi[:], a_hi_ps[:])
                for m0, msz in ((0, 128), (128, 96)):
                    acc = psG.tile([msz, NSL], fp32)
                    nc.tensor.matmul(acc[:], a_lo[:, m0:m0 + msz], a_lo[:],
                                     start=True, stop=False)
                    nc.tensor.matmul(acc[:], a_hi[:, m0:m0 + msz], a_hi[:],
                                     start=False, stop=True)
                    g_sb = g_pool.tile([msz, NSL], fp32)
                    nc.vector.tensor_copy(g_sb[:], acc[:])
                    nc.sync.dma_start(gram_d[b, m0:m0 + msz, :], g_sb[:])

    nc.finalize()
    _CACHE["nc"] = nc
    return nc


def _host_inputs():
    if "host" in _CACHE:
        return _CACHE["host"]
    lt = np.triu(np.ones((128, 128), np.float32), 1)
    iota = np.broadcast_to(np.arange(NSL, dtype=np.float32),
                           (128, NSL)).copy()
    _CACHE["host"] = {
        "lt": lt,
        "ones128": np.ones((128, 1), np.float32),
        "ones1": np.ones((1, 1), np.float32),
        "onesrow": np.ones((1, 128), np.float32),
        "iota224": iota,
    }
    return _CACHE["host"]


def _epilogue(gram: np.ndarray) -> np.ndarray:
    """Reference contrastive loss from per-image pixel Gram matrices.

    gram: [B, 224, 224] f32; valid anchors at [0:100] (class 1) and
    [112:212] (class 2). Mirrors reference ops in f32 (incl. 0*inf=nan)."""
    keep = np.r_[0:N_VIEW, SLOTS:SLOTS + N_VIEW]
    g = gram[:, keep][:, :, keep].astype(np.float32)  # [B, 200, 200]
    lab = np.repeat(np.asarray(CLASSES, np.float32), N_VIEW)
    pos = (lab[:, None] == lab[None, :]).astype(np.float32)
    n = 2 * N_VIEW
    with np.errstate(all="ignore"):
        logits = (g / np.float32(TEMP)).astype(np.float32)
        logits = logits - logits.max(axis=2, keepdims=True)
        mask = pos * (1.0 - np.eye(n, dtype=np.float32))
        exp_logits = np.exp(logits)
        neg_sum = (exp_logits * (1.0 - pos)).sum(axis=2, keepdims=True,
                                                 dtype=np.float32)
        log_prob = logits - np.log(exp_logits + neg_sum)
        mlpp = (mask * log_prob).sum(axis=2) / mask.sum(axis=1)
        losses = -(TEMP / BASE_TEMP) * mlpp.mean(axis=1)
        out = losses.mean()
    return np.float32(out)


def kernel(feats, mask_within, mask_boundary, predict, sel_noise):
    nc = _build_kernel()
    consts = _host_inputs()
    feats = np.asarray(feats, dtype=np.float32)
    in_maps = []
    for core in range(N_CORES):
        lo = core * IMGS_PER_CORE
        hi = lo + IMGS_PER_CORE
        ft = np.ascontiguousarray(
            feats[lo:hi].reshape(IMGS_PER_CORE, C, H, W)
            .transpose(0, 3, 2, 1)).reshape(IMGS_PER_CORE, HW, C)
        in_maps.append({
            "featsT": ft,
            "mask": np.ascontiguousarray(mask_within[lo:hi], dtype=np.int32),
            "noise": np.ascontiguousarray(
                sel_noise[lo:hi], dtype=np.float32).reshape(
                    IMGS_PER_CORE, H, W),
            **consts,
        })
    res = run_bass_kernel_spmd(nc, in_maps, list(range(N_CORES)),
                               **_CACHE.get("run_kwargs", {}))
    _CACHE["last_res"] = res
    grams = np.concatenate([r["gram"] for r in res.results], axis=0)
    return np.asarray(_epilogue(grams))
